# revision 49
# baseline (speedup 1.0000x reference)
"""Trainium2 Bass kernel for nn_DK_50414326120800 (dense_cnn, 8 cores).

Data-parallel over batch: 16 batches -> 2 per NeuronCore. Train-mode
BatchNorm statistics are exchanged with four tiny per-branch AllGather
collectives (15us fixed cost each), overlapped with compute.

Engine assignment (channels on partitions, 2 chunks of 128; pixels free):
  PE    : conv_r / conv_b (bf16 matmuls, fp32 PSUM), ker-gen, and the
          dynamic 4x4 grouped convs (16 diagonal matmuls per chunk
          accumulating shifted APs in PSUM); four chunks hand their
          trailing taps to DVE (see HALF_DVE) and merge with one add.
  DVE   : pooling (one bf16 pairwise fold at the 2x perf mode + exact
          fp32 reduces), BN1 pad builds (banded affine+relu into
          zero-bordered 67x68 images), diag builds, sumsq stats
          (scalar_tensor_tensor is DVE-only on the real ISA and has no
          fast mode), ker-gen bias, most final applies, the HALF_DVE
          trailing-tap FMA chains.
  ACT   : PSUM evictions (y1/guide/y2) with fused accum_out sums, some
          final applies, the Sqrt in the rsqrt chains.
  Pool  : pad border memsets, stat ladders + BN coefficient chains
          (TT/TS-immediate only; TensorScalarPtr is illegal on Pool),
          the collectives (gpsimd queue per NRT straight-line rule).
  SP    : bulk x/out DMAs ordered for earliest consumer (one DMA per
          merged weight pair; images in halves), collective staging.

Conv biases are dropped (they cancel exactly under train-mode BN); the
pooling 1/256 mean factor is folded into kernel-generator weights
host-side.
"""

import sys
from contextlib import ExitStack

import numpy as np

sys.path.insert(0, "/opt/trn_rl_repo")

import ml_dtypes  # noqa: E402
import concourse.bacc as bacc  # noqa: E402
import concourse.mybir as mybir  # noqa: E402
import concourse.tile as tile  # noqa: E402
from concourse.bass_utils import run_bass_kernel_spmd  # noqa: E402

N_CORES = 8
B, CI, C, H, W = 16, 256, 256, 64, 64
BL = B // N_CORES            # local batches per core = 2
NK = 2                       # channel chunks of 128
PIX = H * W                  # 4096
FS = 4
EPS = 1e-5
NTOT = float(B * H * W)      # BN normalizer 65536
HP, WP = 67, 68              # padded image (top2/bot1, left2/right1+1 spare)
F32 = mybir.dt.float32
BF16 = mybir.dt.bfloat16
AF = mybir.ActivationFunctionType
OP = mybir.AluOpType

_CACHE = {}

# dynamic-conv chunks computed on DVE instead of PE (im, k).
# NOTE: scalar_tensor_tensor (3-operand FMA) runs at 1x on DVE (no perf
# modes), so DVE tap-chains cost ~68us/chunk vs 27us on PE — keep empty.
DVE_DYN = set()
# chunks whose dyn conv is split: (im, k) -> number of trailing taps
# computed on DVE (the rest stay on PE)
HALF_DVE = {(1, 1): 8, (3, 1): 4, (1, 0): 4, (0, 1): 4, (0, 0): 4}


def build(debug=False):
    nc = bacc.Bacc("TRN2", target_bir_lowering=False, num_devices=N_CORES)

    # ---- DRAM I/O --------------------------------------------------------
    xf_d = nc.dram_tensor("xf", [BL, NK, 128, PIX], BF16, kind="ExternalInput")
    xe_d = nc.dram_tensor("xe", [BL, NK, 128, PIX], BF16, kind="ExternalInput")
    w_in = {}
    for nm in ["wrf", "wre", "wbf", "wbe"]:
        w_in[nm] = nc.dram_tensor(nm, [NK, 128, C], BF16,
                                  kind="ExternalInput")
    for nm in ["wkf", "wke"]:
        w_in[nm] = nc.dram_tensor(nm, [NK, 128, C], F32,
                                  kind="ExternalInput")
    bkf_d = nc.dram_tensor("bkf", [128, 2], F32, kind="ExternalInput")
    bke_d = nc.dram_tensor("bke", [128, 2], F32, kind="ExternalInput")
    g1p_d = nc.dram_tensor("g1p", [128, 4], F32, kind="ExternalInput")
    be1p_d = nc.dram_tensor("be1p", [128, 4], F32, kind="ExternalInput")
    g2p_d = nc.dram_tensor("g2p", [128, 4], F32, kind="ExternalInput")
    be2p_d = nc.dram_tensor("be2p", [128, 4], F32, kind="ExternalInput")
    id_d = nc.dram_tensor("identbf", [128, 128], BF16, kind="ExternalInput")
    gf_d = nc.dram_tensor("gf", [BL, NK, 128, PIX], BF16,
                          kind="ExternalOutput")
    ge_d = nc.dram_tensor("ge", [BL, NK, 128, PIX], BF16,
                          kind="ExternalOutput")

    with tile.TileContext(nc) as tc, ExitStack() as ctx:
        cpool = ctx.enter_context(tc.tile_pool(name="consts", bufs=1))
        xpool = ctx.enter_context(tc.tile_pool(name="xin", bufs=4))
        imgpool = ctx.enter_context(tc.tile_pool(name="img", bufs=7))
        padpool = ctx.enter_context(tc.tile_pool(name="pads", bufs=5))
        gpool = ctx.enter_context(tc.tile_pool(name="guide", bufs=4))
        opool = ctx.enter_context(tc.tile_pool(name="outst", bufs=2))
        scrpool = ctx.enter_context(tc.tile_pool(name="scrp", bufs=2))
        dpool = ctx.enter_context(tc.tile_pool(name="diags", bufs=16))
        spool = ctx.enter_context(tc.tile_pool(name="small", bufs=1))
        pspool = ctx.enter_context(tc.tile_pool(name="ps", bufs=2, space="PSUM"))
        drpool = ctx.enter_context(tc.tile_pool(name="drb", bufs=1, space="DRAM"))

        # ---- DMA preamble (SP queue), ordered for earliest consumer ----
        wt = {}

        def load_w(nm, dt_):
            # one tile + one DMA for both k-chunks (HWDGE overhead is
            # ~0.6us per dma_start; halve the count)
            t = cpool.tile([128, 2 * C], dt_, name=f"sb_{nm}", tag=f"sb_{nm}")
            nc.sync.dma_start(
                out=t.rearrange("p (k c) -> p k c", k=NK, c=C),
                in_=w_in[nm].rearrange("k p c -> p k c"))
            wt[nm] = t

        xdram = {0: xf_d, 1: xe_d}
        xt_all = {}           # (im, k) -> tile;  im = b*2 + br

        def load_x(b, br, quarters):
            im = b * 2 + br
            for k in range(NK):
                xt_all[(im, k)] = xpool.tile(
                    [128, PIX], BF16, name=f"x_{im}_{k}", tag="x")
            if quarters:
                for s in range(2):
                    sl = slice(s * 2048, (s + 1) * 2048)
                    for k in range(NK):
                        nc.sync.dma_start(out=xt_all[(im, k)][:, sl],
                                          in_=xdram[br][b, k][:, sl])
            else:
                for k in range(NK):
                    nc.sync.dma_start(out=xt_all[(im, k)][:, :],
                                      in_=xdram[br][b, k])

        # wrf m0-columns first: the very first matmul needs only them
        wrf_t = cpool.tile([128, 2 * C], BF16, name="sb_wrf", tag="sb_wrf")
        wrf3 = wrf_t.rearrange("p (k c) -> p k c", k=NK, c=C)
        win3 = w_in["wrf"].rearrange("k p c -> p k c")
        nc.sync.dma_start(out=wrf3[:, :, 0:128], in_=win3[:, :, 0:128])
        wt["wrf"] = wrf_t
        load_x(0, 0, quarters=True)       # im0
        nc.sync.dma_start(out=wrf3[:, :, 128:256], in_=win3[:, :, 128:256])
        load_x(1, 0, quarters=False)      # im2
        load_w("wre", BF16)
        load_x(0, 1, quarters=False)      # im1
        load_x(1, 1, quarters=False)      # im3
        load_w("wke", F32)
        bk_sb = {}
        for nm, d in [("bkf", bkf_d), ("bke", bke_d)]:
            t = cpool.tile([128, 2], F32, name=f"sb_{nm}", tag=f"sb_{nm}")
            nc.sync.dma_start(out=t[:, :], in_=d[:, :])
            bk_sb[nm] = t
        packs = {}
        for nm, d in [("g1p", g1p_d), ("be1p", be1p_d), ("g2p", g2p_d),
                      ("be2p", be2p_d)]:
            t = cpool.tile([128, 4], F32, name=f"sb_{nm}", tag=f"sb_{nm}")
            nc.sync.dma_start(out=t[:, :], in_=d[:, :])
            packs[nm] = t
        ident = cpool.tile([128, 128], BF16, name="sb_ident", tag="sb_ident")
        nc.sync.dma_start(out=ident[:, :], in_=id_d[:, :])
        load_w("wkf", F32)
        load_w("wbf", BF16)
        load_w("wbe", BF16)

        pooled = {}
        kers = {}
        for b in range(BL):
            for br in range(2):
                for k in range(NK):
                    pooled[(b, br, k)] = spool.tile(
                        [128, 16], F32, name=f"pool_{b}_{br}_{k}", tag="pooled",
                        bufs=BL * 2 * NK)
                    kers[(b, br, k)] = spool.tile(
                        [128, 16], F32, name=f"ker_{b}_{br}_{k}", tag="kers",
                        bufs=BL * 2 * NK)

        y1 = {}
        y2 = {}

        # ---- pooling: 16x16 block sums, all DVE. One bf16 pairwise fold
        # at the 2x perf mode (error ~0.2% on pooled sums, negligible
        # after the kernel-generator averaging), then exact fp32 reduces.
        def do_pool(b, br, k):
            im = b * 2 + br
            xt = xt_all[(im, k)]
            x3 = xt.rearrange("p (yx xi) -> p yx xi", yx=256, xi=16)
            t1 = spool.tile([128, 2048], BF16, name=f"t1_{im}_{k}",
                            tag="poolt1", bufs=1)
            t13 = t1.rearrange("p (yx xh) -> p yx xh", yx=256, xh=8)
            nc.vector.tensor_tensor(out=t13, in0=x3[:, :, 0:8],
                                    in1=x3[:, :, 8:16], op=OP.add)
            s1 = spool.tile([128, 256], F32, name=f"s1_{im}_{k}",
                            tag="s1", bufs=2)
            nc.vector.tensor_reduce(
                out=s1[:, :], in_=t13, axis=mybir.AxisListType.X, op=OP.add)
            # s1 layout [(y64, xb4)]; reduce y-inner 16 (strided)
            s2in = s1.rearrange("p (yb yi xb) -> p yb xb yi", yb=4, yi=16,
                                xb=4)
            nc.vector.tensor_reduce(
                out=pooled[(b, br, k)].rearrange(
                    "p (yb xb) -> p yb xb", yb=4, xb=4),
                in_=s2in, axis=mybir.AxisListType.X, op=OP.add)

        def do_kergen(b, br):
            knm = "wkf" if br == 0 else "wke"
            bnm = "bkf" if br == 0 else "bke"
            for m in range(NK):
                kps = pspool.tile([128, 1024], F32, name=f"kgp_{b}_{br}_{m}",
                                  tag="dynps", bufs=2)
                for k in range(NK):
                    nc.tensor.matmul(
                        kps[:, 0:16],
                        wt[knm][:, k * C + m * 128:k * C + (m + 1) * 128],
                        pooled[(b, br, k)][:, :],
                        start=(k == 0), stop=(k == NK - 1))
                # bias on DVE: the ACT queue is deep in y1 evicts here and
                # kers gate the diag builds
                nc.vector.tensor_scalar(
                    out=kers[(b, br, m)][:, :], in0=kps[:, 0:16],
                    scalar1=bk_sb[bnm][:, m:m + 1], scalar2=None, op0=OP.add)

        # ---- conv_r / conv_b: PE matmuls + ACT evict w/ accum sums.
        # sumsq reads the PSUM tile directly on DVE, in parallel with the
        # ACT eviction, so branch stats complete right after the last
        # matmul instead of serializing behind the bf16 eviction. ----
        def conv1x1_pe(im, src, wnm, ydict, scr, ssq=None, ssq_psum=False):
            b = im // 2
            for m in range(NK):
                yt = imgpool.tile([128, PIX], BF16, name=f"y_{wnm}_{im}_{m}",
                                  tag="img")
                ydict[(im, m)] = yt
                for q in range(4):
                    mp = pspool.tile([128, 1024], F32,
                                     name=f"mp_{wnm}_{im}_{m}_{q}", tag="mmps",
                                     bufs=2)
                    for n in range(2):
                        off = q * 1024 + n * 512
                        for k in range(NK):
                            nc.tensor.matmul(
                                mp[:, n * 512:(n + 1) * 512],
                                wt[wnm][:, k * C + m * 128:
                                         k * C + (m + 1) * 128],
                                src[k][:, off:off + 512],
                                start=(k == 0), stop=(k == NK - 1))
                    g = q * 4 + b * 2 + m
                    sl = slice(q * 1024, (q + 1) * 1024)
                    nc.scalar.activation(
                        yt[:, sl], mp[:, :], AF.Copy,
                        accum_out=scr[:, g:g + 1])
                    if ssq is not None:
                        jk = scrpool.tile([128, 1024], BF16,
                                          name=f"jkp_{wnm}_{im}_{m}_{q}",
                                          tag="scr")
                        sqin = mp[:, :] if ssq_psum else yt[:, sl]
                        nc.vector.scalar_tensor_tensor(
                            out=jk[:, :], in0=sqin, scalar=1.0,
                            in1=sqin, op0=OP.mult, op1=OP.mult,
                            accum_out=ssq[:, g:g + 1])

        def sumsq_ops(im, yt_dict, scr_ssq, eng="dve"):
            b = im // 2
            for m in range(NK):
                yt = yt_dict[(im, m)]
                for q in range(4):
                    g = q * 4 + b * 2 + m
                    sl = slice(q * 1024, (q + 1) * 1024)
                    jk = scrpool.tile([128, 1024], BF16,
                                      name=f"jk_{id(scr_ssq)}_{im}_{m}_{q}",
                                      tag="scr")
                    # AP-scalar ops (TensorScalarPtr/STT) are DVE-only on
                    # the real ISA; the Pool engine rejects them.
                    nc.vector.scalar_tensor_tensor(
                        out=jk[:, :], in0=yt[:, sl], scalar=1.0,
                        in1=yt[:, sl], op0=OP.mult, op1=OP.mult,
                        accum_out=scr_ssq[:, g:g + 1])

        # ---- stat ladders (DVE) + collective (cin/gth on SP, cc on Pool) --
        def reduce16_to4(t16, dst, pfx):
            s8 = spool.tile([128, 8], F32, name=f"s8{pfx}", tag=f"s8{pfx}")
            nc.gpsimd.tensor_tensor(out=s8[:, :], in0=t16[:, 0:8],
                                    in1=t16[:, 8:16], op=OP.add)
            nc.gpsimd.tensor_tensor(out=dst, in0=s8[:, 0:4],
                                    in1=s8[:, 4:8], op=OP.add)

        def reduce_stats(scr, ssq, pfx):
            sums = spool.tile([128, 4], F32, name=f"sums{pfx}", tag=f"sums{pfx}")
            reduce16_to4(scr, sums[:, :], f"a{pfx}")
            sq4 = spool.tile([128, 4], F32, name=f"sq4{pfx}", tag=f"sq4{pfx}")
            reduce16_to4(ssq, sq4[:, :], f"b{pfx}")
            loc = spool.tile([128, 4], F32, name=f"loc{pfx}", tag=f"loc{pfx}")
            nc.gpsimd.tensor_tensor(out=loc[:, 0:2], in0=sums[:, 0:2],
                                    in1=sums[:, 2:4], op=OP.add)
            nc.gpsimd.tensor_tensor(out=loc[:, 2:4], in0=sq4[:, 0:2],
                                    in1=sq4[:, 2:4], op=OP.add)
            return loc

        cc_state = {}

        def cc_launch(pfx, loc):
            """stage local stats to DRAM (SP queue) + AllGather (Pool)."""
            cin = drpool.tile([128, 4], F32, name=f"{pfx}i", tag=f"{pfx}i")
            cout = drpool.tile([N_CORES, 128, 4], F32, name=f"{pfx}o",
                               tag=f"{pfx}o", addr_space="Shared")
            nc.sync.dma_start(out=cin[:, :], in_=loc[:, :])
            nc.gpsimd.collective_compute(
                "AllGather", OP.bypass,
                replica_groups=[list(range(N_CORES))],
                ins=[cin[:, :]], outs=[cout[:, :, :]])
            cc_state[pfx] = cout

        def cc_finish(pfx):
            """gather back (SP) + pairwise reduce over cores (DVE)."""
            cout = cc_state[pfx]
            gth = spool.tile([128, 32], F32, name=f"{pfx}g", tag=f"{pfx}g")
            nc.sync.dma_start(
                out=gth.rearrange("p (n f) -> p n f", n=N_CORES, f=4),
                in_=cout.rearrange("n p f -> p n f"))
            t16 = spool.tile([128, 16], F32, name=f"{pfx}h", tag=f"{pfx}h")
            nc.gpsimd.tensor_tensor(out=t16[:, :], in0=gth[:, 0:16],
                                    in1=gth[:, 16:32], op=OP.add)
            t8 = spool.tile([128, 8], F32, name=f"{pfx}q", tag=f"{pfx}q")
            nc.gpsimd.tensor_tensor(out=t8[:, :], in0=t16[:, 0:8],
                                    in1=t16[:, 8:16], op=OP.add)
            g = spool.tile([128, 4], F32, name=f"{pfx}r", tag=f"{pfx}r")
            nc.gpsimd.tensor_tensor(out=g[:, :], in0=t8[:, 0:4],
                                    in1=t8[:, 4:8], op=OP.add)
            return g

        def bn_coeffs(gst, gpack, bepack, pfx, eng="pool"):
            """global (sum, sumsq) [128,4] -> scale/shift [128,2]; Pool by
            default (off the contended DVE), DVE for the tail where DVE is
            idle. reciprocal + ACT Sqrt + one Newton step refine rsqrt."""
            ve = nc.gpsimd if eng == "pool" else nc.vector
            mean = spool.tile([128, 2], F32, name=f"{pfx}_mean", tag=f"{pfx}_mean")
            ve.tensor_scalar(out=mean[:, :], in0=gst[:, 0:2],
                             scalar1=1.0 / NTOT, scalar2=None, op0=OP.mult)
            vpe = spool.tile([128, 2], F32, name=f"{pfx}_vpe", tag=f"{pfx}_vpe")
            ve.tensor_scalar(out=vpe[:, :], in0=gst[:, 2:4],
                             scalar1=1.0 / NTOT, scalar2=EPS,
                             op0=OP.mult, op1=OP.add)
            msq = spool.tile([128, 2], F32, name=f"{pfx}_msq", tag=f"{pfx}_msq")
            ve.tensor_tensor(out=msq[:, :], in0=mean[:, :],
                             in1=mean[:, :], op=OP.mult)
            ve.tensor_tensor(out=vpe[:, :], in0=vpe[:, :],
                             in1=msq[:, :], op=OP.subtract)
            rcp = spool.tile([128, 2], F32, name=f"{pfx}_rcp", tag=f"{pfx}_rcp")
            nc.vector.reciprocal(rcp[:, :], vpe[:, :])
            r0 = spool.tile([128, 2], F32, name=f"{pfx}_r0", tag=f"{pfx}_r0")
            nc.scalar.activation(r0[:, :], rcp[:, :], AF.Sqrt)
            t1 = spool.tile([128, 2], F32, name=f"{pfx}_t1", tag=f"{pfx}_t1")
            ve.tensor_tensor(out=t1[:, :], in0=r0[:, :], in1=r0[:, :],
                             op=OP.mult)
            ve.tensor_tensor(out=t1[:, :], in0=vpe[:, :], in1=t1[:, :],
                             op=OP.mult)
            ve.tensor_scalar(out=t1[:, :], in0=t1[:, :], scalar1=-0.5,
                             scalar2=1.5, op0=OP.mult, op1=OP.add)
            ve.tensor_tensor(out=r0[:, :], in0=r0[:, :], in1=t1[:, :],
                             op=OP.mult)
            sc = spool.tile([128, 2], F32, name=f"{pfx}_sc", tag=f"{pfx}_sc")
            ve.tensor_tensor(out=sc[:, :], in0=gpack[:, :],
                             in1=r0[:, :], op=OP.mult)
            sh = spool.tile([128, 2], F32, name=f"{pfx}_sh", tag=f"{pfx}_sh")
            ve.tensor_tensor(out=sh[:, :], in0=mean[:, :],
                             in1=sc[:, :], op=OP.mult)
            ve.tensor_tensor(out=sh[:, :], in0=bepack[:, :],
                             in1=sh[:, :], op=OP.subtract)
            return sc, sh

        # ================= PHASE A =======================================
        scrA = {}
        ssqA = {}
        for br in range(2):
            scrA[br] = spool.tile([128, 16], F32, name=f"scrA{br}", tag=f"scrA{br}")
            ssqA[br] = spool.tile([128, 16], F32, name=f"ssqA{br}", tag=f"ssqA{br}")

        # br0 stats chain gates the first collective: split its sumsq over
        # DVE+Pool and launch cc10 as soon as im0/im2 y1 evicts land.
        # im2's pooling + br0 kergen are deferred into phase B (kers_f are
        # first needed by im1's diags, much later); br1's pooling runs
        # right after the cc10 launch so kergen_e -> im0 diags unblock.
        for k in range(NK):
            do_pool(0, 0, k)
        conv1x1_pe(0, {k: xt_all[(0, k)] for k in range(NK)},
                   "wrf", y1, scrA[0], ssq=ssqA[0])
        conv1x1_pe(2, {k: xt_all[(2, k)] for k in range(NK)},
                   "wrf", y1, scrA[0], ssq=ssqA[0])
        cc_launch("cc10", reduce_stats(scrA[0], ssqA[0], "A0"))

        for b in range(BL):
            for k in range(NK):
                do_pool(b, 1, k)
        conv1x1_pe(1, {k: xt_all[(1, k)] for k in range(NK)},
                   "wre", y1, scrA[1])
        conv1x1_pe(3, {k: xt_all[(3, k)] for k in range(NK)},
                   "wre", y1, scrA[1])
        for b in range(BL):
            do_kergen(b, 1)

        # ================= PHASE B =======================================
        outdram = {0: gf_d, 1: ge_d}
        sc1 = {}
        sh1 = {}
        sc2 = {}
        sh2 = {}

        def make_pad(b, br, k):
            """zero-bordered BN1+ReLU image chunk; DVE affine+relu interior,
            Pool border memsets."""
            im = b * 2 + br
            pt = padpool.tile([128, HP * WP], BF16, name=f"pad_{im}_{k}",
                              tag="pad")
            p3 = pt.rearrange("p (h w) -> p h w", h=HP, w=WP)
            nc.gpsimd.memset(p3[:, 0:2, :], 0.0)      # top 2 rows
            nc.gpsimd.memset(p3[:, 66:67, :], 0.0)    # bottom row
            nc.gpsimd.memset(p3[:, 2:66, 0:2], 0.0)   # left 2 cols
            nc.gpsimd.memset(p3[:, 2:66, 66:68], 0.0)  # right 2 cols
            # two row bands: the first dyn matmuls start after band A
            y3 = y1[(im, k)].rearrange("p (h w) -> p h w", h=64, w=64)
            for r0, r1 in ((0, 33), (33, 64)):
                band = p3[:, 2 + r0:2 + r1, 2:66]
                nc.vector.tensor_scalar(
                    out=band, in0=y3[:, r0:r1, :],
                    scalar1=sc1[br][:, k:k + 1], scalar2=sh1[br][:, k:k + 1],
                    op0=OP.mult, op1=OP.add)
                nc.vector.tensor_scalar(
                    out=band, in0=band, scalar1=0.0, scalar2=None,
                    op0=OP.max)
            return p3

        def make_diags(b, br, k, taps=range(16)):
            """diagonal tap matrices from the OTHER branch's kernels (DVE)."""
            im = b * 2 + br
            kt = kers[(b, 1 - br, k)]
            dg = {}
            for t in taps:
                dt_ = dpool.tile([128, 128], BF16, name=f"dg_{im}_{k}_{t}",
                                 tag="diag")
                nc.vector.tensor_scalar(
                    out=dt_[:, :], in0=ident[:, :],
                    scalar1=kt[:, t:t + 1], scalar2=None, op0=OP.mult)
                dg[t] = dt_
            return dg

        def dyn_pe(im, k, p3, dg, taps=range(16)):
            """grouped conv on PE: diag matmuls over shifted APs."""
            gt = gpool.tile([128, PIX], BF16, name=f"gd_{im}_{k}", tag="guide")
            taps = list(taps)
            for q in range(4):
                dp = pspool.tile([128, 1024], F32, name=f"dp_{im}_{k}_{q}",
                                 tag="dynps", bufs=2)
                for t in taps:
                    i, j = t // 4, t % 4
                    for n in range(2):
                        r0_ = q * 16 + n * 8 + i
                        nc.tensor.matmul(
                            dp[:, n * 512:(n + 1) * 512],
                            dg[t][:, :],
                            p3[:, r0_:r0_ + 8, j:j + 64],
                            start=(t == taps[0]), stop=(t == taps[-1]))
                nc.scalar.activation(
                    gt[:, q * 1024:(q + 1) * 1024], dp[:, :], AF.Copy)
            return gt

        def dyn_dve(im, k, p3, b, br, taps=range(16)):
            """grouped-conv tap subset as a DVE FMA chain (1x, slow — use
            only to shave the PE critical path with idle DVE time)."""
            kt = kers[(b, 1 - br, k)]
            taps = list(taps)
            gt = gpool.tile([128, PIX], BF16, name=f"gdv_{im}_{k}",
                            tag="guide")
            g3 = gt.rearrange("p (h w) -> p h w", h=64, w=64)
            for t in taps:
                i, j = t // 4, t % 4
                win = p3[:, i:i + 64, j:j + 64]
                if t == taps[0]:
                    nc.vector.tensor_scalar(
                        out=g3, in0=win, scalar1=kt[:, t:t + 1], scalar2=None,
                        op0=OP.mult)
                else:
                    nc.vector.scalar_tensor_tensor(
                        out=g3, in0=win, scalar=kt[:, t:t + 1], in1=g3,
                        op0=OP.mult, op1=OP.add)
            return gt

        def final_apply(b, br, m, engine):
            """BN2+ReLU -> bf16 -> DMA out, quartered for DMA pipelining."""
            im = b * 2 + br
            ot = opool.tile([128, PIX], BF16, name=f"o_{im}_{m}", tag="outb")
            for h in range(2):
                sl = slice(h * 2048, (h + 1) * 2048)
                if engine == "dve":
                    nc.vector.tensor_scalar(
                        out=ot[:, sl], in0=y2[(im, m)][:, sl],
                        scalar1=sc2[br][:, m:m + 1],
                        scalar2=sh2[br][:, m:m + 1],
                        op0=OP.mult, op1=OP.add)
                    nc.vector.tensor_scalar(
                        out=ot[:, sl], in0=ot[:, sl], scalar1=0.0,
                        scalar2=None, op0=OP.max)
                else:
                    nc.scalar.activation(
                        ot[:, sl], y2[(im, m)][:, sl], AF.Relu,
                        bias=sh2[br][:, m:m + 1],
                        scale=sc2[br][:, m:m + 1])
                nc.sync.dma_start(out=outdram[br][b, m][:, sl],
                                  in_=ot[:, sl])

        scrB = {}
        ssqB = {}
        for br in range(2):
            scrB[br] = spool.tile([128, 16], F32, name=f"scrB{br}", tag=f"scrB{br}")
            ssqB[br] = spool.tile([128, 16], F32, name=f"ssqB{br}", tag=f"ssqB{br}")

        def image_phaseB(b, br, dg0=None, ssq_psum=False):
            """pads + dyn + conv_b for one image."""
            im = b * 2 + br
            guides = {}
            for k in range(NK):
                p3 = make_pad(b, br, k)
                if (im, k) in HALF_DVE:
                    # leading taps on PE, trailing taps on DVE, one-add merge
                    nd = HALF_DVE[(im, k)]
                    dg = make_diags(b, br, k, taps=range(16 - nd))
                    ga = dyn_pe(im, k, p3, dg, taps=range(16 - nd))
                    gb = dyn_dve(im, k, p3, b, br, taps=range(16 - nd, 16))
                    nc.vector.tensor_tensor(out=ga[:, :], in0=ga[:, :],
                                            in1=gb[:, :], op=OP.add)
                    guides[k] = ga
                elif (im, k) in DVE_DYN:
                    guides[k] = dyn_dve(im, k, p3, b, br)
                else:
                    dg = dg0 if (k == 0 and dg0 is not None) \
                        else make_diags(b, br, k)
                    guides[k] = dyn_pe(im, k, p3, dg)
            bnm2 = "wbf" if br == 0 else "wbe"
            conv1x1_pe(im, guides, bnm2, y2, scrB[br], ssq=ssqB[br],
                       ssq_psum=ssq_psum)

        # diags for im0 k0 ahead of the BN1-coeff wait on the DVE queue
        # (they depend only on br1 kers)
        dg00 = make_diags(0, 0, 0)

        gst1_0 = cc_finish("cc10")
        sc1[0], sh1[0] = bn_coeffs(
            gst1_0, packs["g1p"][:, 0:2], packs["be1p"][:, 0:2], "c10")

        # ---- im0 (b0, br0) ----
        image_phaseB(0, 0, dg0=dg00)

        # br1's deferred phase-A stats: DVE is free here, after im0's pads
        sumsq_ops(1, y1, ssqA[1])
        sumsq_ops(3, y1, ssqA[1])
        cc_launch("cc11", reduce_stats(scrA[1], ssqA[1], "A1"))

        # deferred im2 pooling + br0 kergen (kers_f first used by im1)
        for k in range(NK):
            do_pool(1, 0, k)
        for b in range(BL):
            do_kergen(b, 0)

        # ---- im2 (b1, br0) ----
        image_phaseB(1, 0)

        gst1_1 = cc_finish("cc11")
        sc1[1], sh1[1] = bn_coeffs(
            gst1_1, packs["g1p"][:, 2:4], packs["be1p"][:, 2:4], "c11")

        # ---- im1 (b0, br1) ----
        image_phaseB(0, 1)
        cc_launch("cc20", reduce_stats(scrB[0], ssqB[0], "B0"))

        # ---- im3 (b1, br1): sumsq straight from PSUM — the stats gate
        # the tail collective and nothing runs after to be throttled ----
        image_phaseB(1, 1)

        # ---- br0 finals (all DVE; ACT busy trailing br1 evicts) ----
        gst2_0 = cc_finish("cc20")
        sc2[0], sh2[0] = bn_coeffs(
            gst2_0, packs["g2p"][:, 0:2], packs["be2p"][:, 0:2], "c20")
        for b in range(BL):
            for m in range(NK):
                final_apply(b, 0, m, "dve")

        cc_launch("cc21", reduce_stats(scrB[1], ssqB[1], "B1"))

        # ---- br1 finals (split ACT/DVE for a short tail) ----
        gst2_1 = cc_finish("cc21")
        sc2[1], sh2[1] = bn_coeffs(
            gst2_1, packs["g2p"][:, 2:4], packs["be2p"][:, 2:4], "c21",
            eng="dve")
        final_apply(0, 1, 0, "dve")
        final_apply(0, 1, 1, "act")
        final_apply(1, 1, 0, "dve")
        final_apply(1, 1, 1, "dve")

    nc.compile()
    return nc


def _prep_maps(xf, xe, w_kf, b_kf, w_ke, b_ke, w_rf, g_rf, be_rf, w_re, g_re,
               be_re, w_bf, g_bf, be_bf, w_be, g_be, be_be):
    bf = ml_dtypes.bfloat16
    common = {}
    for nm, w, dt_ in [("wrf", w_rf, bf), ("wre", w_re, bf), ("wbf", w_bf, bf),
                       ("wbe", w_be, bf), ("wkf", w_kf / 256.0, np.float32),
                       ("wke", w_ke / 256.0, np.float32)]:
        wT = np.ascontiguousarray(np.asarray(w, np.float32).T.astype(dt_))
        common[nm] = wT.reshape(NK, 128, C)
    common["bkf"] = np.ascontiguousarray(
        np.asarray(b_kf, np.float32).reshape(2, 128).T)
    common["bke"] = np.ascontiguousarray(
        np.asarray(b_ke, np.float32).reshape(2, 128).T)

    def pack(gf_, ge_):
        p = np.zeros((128, 4), np.float32)
        for br in range(2):
            for m in range(NK):
                v = gf_ if br == 0 else ge_
                p[:, br * 2 + m] = np.asarray(v, np.float32)[
                    m * 128:(m + 1) * 128]
        return p

    common["g1p"] = pack(g_rf, g_re)
    common["be1p"] = pack(be_rf, be_re)
    common["g2p"] = pack(g_bf, g_be)
    common["be2p"] = pack(be_bf, be_be)
    common["identbf"] = np.eye(128, dtype=np.float32).astype(bf)

    xf = np.asarray(xf, np.float32).reshape(N_CORES, BL, NK, 128, PIX)
    xe = np.asarray(xe, np.float32).reshape(N_CORES, BL, NK, 128, PIX)
    maps = []
    for c in range(N_CORES):
        m = dict(common)
        m["xf"] = xf[c].astype(bf)
        m["xe"] = xe[c].astype(bf)
        maps.append(m)
    return maps


def kernel(xf, xe, w_kf, b_kf, w_ke, b_ke,
           w_rf, b_rf, g_rf, be_rf, w_re, b_re, g_re, be_re,
           w_bf, b_bf, g_bf, be_bf, w_be, b_be, g_be, be_be):
    # note: conv biases feeding a train-mode BatchNorm cancel exactly
    # (BN subtracts the batch mean), so b_rf/b_re/b_bf/b_be are unused.
    try:
        import jax
        jax.config.update("jax_compilation_cache_dir", "/tmp/jaxcache_kernel")
        jax.config.update("jax_persistent_cache_min_entry_size_bytes", 0)
        jax.config.update("jax_persistent_cache_min_compile_time_secs", 0)
    except Exception:
        pass
    if "nc" not in _CACHE:
        _CACHE["nc"] = build()
    nc = _CACHE["nc"]
    maps = _prep_maps(xf, xe, w_kf, b_kf, w_ke, b_ke, w_rf, g_rf, be_rf,
                      w_re, g_re, be_re, w_bf, g_bf, be_bf, w_be, g_be, be_be)
    res = run_bass_kernel_spmd(nc, maps, core_ids=list(range(N_CORES)))
    gf = np.concatenate(
        [np.asarray(r["gf"]).astype(np.float32).reshape(BL, C, H, W)
         for r in res.results])
    ge = np.concatenate(
        [np.asarray(r["ge"]).astype(np.float32).reshape(BL, C, H, W)
         for r in res.results])
    return gf, ge


# revision 50
# speedup vs baseline: 1.0035x; 1.0035x over previous
"""Trainium2 Bass kernel for nn_DK_50414326120800 (dense_cnn, 8 cores).

Data-parallel over batch: 16 batches -> 2 per NeuronCore. Train-mode
BatchNorm statistics are exchanged with four tiny per-branch AllGather
collectives (15us fixed cost each), overlapped with compute.

Engine assignment (channels on partitions, 2 chunks of 128; pixels free):
  PE    : conv_r / conv_b (bf16 matmuls, fp32 PSUM), ker-gen, and the
          dynamic 4x4 grouped convs (16 diagonal matmuls per chunk
          accumulating shifted APs in PSUM); four chunks hand their
          trailing taps to DVE (see HALF_DVE) and merge with one add.
  DVE   : pooling (one bf16 pairwise fold at the 2x perf mode + exact
          fp32 reduces), BN1 pad builds (banded affine+relu into
          zero-bordered 67x68 images), diag builds, sumsq stats
          (scalar_tensor_tensor is DVE-only on the real ISA and has no
          fast mode), ker-gen bias, most final applies, the HALF_DVE
          trailing-tap FMA chains.
  ACT   : PSUM evictions (y1/guide/y2) with fused accum_out sums, some
          final applies, the Sqrt in the rsqrt chains.
  Pool  : pad border memsets, stat ladders + BN coefficient chains
          (TT/TS-immediate only; TensorScalarPtr is illegal on Pool),
          the collectives (gpsimd queue per NRT straight-line rule).
  SP    : bulk x/out DMAs ordered for earliest consumer (one DMA per
          merged weight pair; images in halves), collective staging.

Conv biases are dropped (they cancel exactly under train-mode BN); the
pooling 1/256 mean factor is folded into kernel-generator weights
host-side.
"""

import sys
from contextlib import ExitStack

import numpy as np

sys.path.insert(0, "/opt/trn_rl_repo")

import ml_dtypes  # noqa: E402
import concourse.bacc as bacc  # noqa: E402
import concourse.mybir as mybir  # noqa: E402
import concourse.tile as tile  # noqa: E402
from concourse.bass_utils import run_bass_kernel_spmd  # noqa: E402

N_CORES = 8
B, CI, C, H, W = 16, 256, 256, 64, 64
BL = B // N_CORES            # local batches per core = 2
NK = 2                       # channel chunks of 128
PIX = H * W                  # 4096
FS = 4
EPS = 1e-5
NTOT = float(B * H * W)      # BN normalizer 65536
HP, WP = 67, 68              # padded image (top2/bot1, left2/right1+1 spare)
F32 = mybir.dt.float32
BF16 = mybir.dt.bfloat16
AF = mybir.ActivationFunctionType
OP = mybir.AluOpType

_CACHE = {}

# dynamic-conv chunks computed on DVE instead of PE (im, k).
# NOTE: scalar_tensor_tensor (3-operand FMA) runs at 1x on DVE (no perf
# modes), so DVE tap-chains cost ~68us/chunk vs 27us on PE — keep empty.
DVE_DYN = set()
# chunks whose dyn conv is split: (im, k) -> number of trailing taps
# computed on DVE (the rest stay on PE)
HALF_DVE = {(1, 1): 8, (3, 1): 4, (1, 0): 4, (0, 1): 4, (0, 0): 4}


def build(debug=False):
    nc = bacc.Bacc("TRN2", target_bir_lowering=False, num_devices=N_CORES)

    # ---- DRAM I/O --------------------------------------------------------
    xf_d = nc.dram_tensor("xf", [BL, NK, 128, PIX], BF16, kind="ExternalInput")
    xe_d = nc.dram_tensor("xe", [BL, NK, 128, PIX], BF16, kind="ExternalInput")
    w_in = {}
    for nm in ["wrf", "wre", "wbf", "wbe"]:
        w_in[nm] = nc.dram_tensor(nm, [NK, 128, C], BF16,
                                  kind="ExternalInput")
    for nm in ["wkf", "wke"]:
        w_in[nm] = nc.dram_tensor(nm, [NK, 128, C], F32,
                                  kind="ExternalInput")
    bkf_d = nc.dram_tensor("bkf", [128, 2], F32, kind="ExternalInput")
    bke_d = nc.dram_tensor("bke", [128, 2], F32, kind="ExternalInput")
    g1p_d = nc.dram_tensor("g1p", [128, 4], F32, kind="ExternalInput")
    be1p_d = nc.dram_tensor("be1p", [128, 4], F32, kind="ExternalInput")
    g2p_d = nc.dram_tensor("g2p", [128, 4], F32, kind="ExternalInput")
    be2p_d = nc.dram_tensor("be2p", [128, 4], F32, kind="ExternalInput")
    id_d = nc.dram_tensor("identbf", [128, 128], BF16, kind="ExternalInput")
    gf_d = nc.dram_tensor("gf", [BL, NK, 128, PIX], BF16,
                          kind="ExternalOutput")
    ge_d = nc.dram_tensor("ge", [BL, NK, 128, PIX], BF16,
                          kind="ExternalOutput")

    with tile.TileContext(nc) as tc, ExitStack() as ctx:
        cpool = ctx.enter_context(tc.tile_pool(name="consts", bufs=1))
        xpool = ctx.enter_context(tc.tile_pool(name="xin", bufs=4))
        imgpool = ctx.enter_context(tc.tile_pool(name="img", bufs=7))
        padpool = ctx.enter_context(tc.tile_pool(name="pads", bufs=5))
        gpool = ctx.enter_context(tc.tile_pool(name="guide", bufs=4))
        opool = ctx.enter_context(tc.tile_pool(name="outst", bufs=2))
        scrpool = ctx.enter_context(tc.tile_pool(name="scrp", bufs=2))
        dpool = ctx.enter_context(tc.tile_pool(name="diags", bufs=16))
        spool = ctx.enter_context(tc.tile_pool(name="small", bufs=1))
        pspool = ctx.enter_context(tc.tile_pool(name="ps", bufs=2, space="PSUM"))
        drpool = ctx.enter_context(tc.tile_pool(name="drb", bufs=1, space="DRAM"))

        # ---- DMA preamble (SP queue), ordered for earliest consumer ----
        wt = {}

        def load_w(nm, dt_):
            # one tile + one DMA for both k-chunks (HWDGE overhead is
            # ~0.6us per dma_start; halve the count)
            t = cpool.tile([128, 2 * C], dt_, name=f"sb_{nm}", tag=f"sb_{nm}")
            nc.sync.dma_start(
                out=t.rearrange("p (k c) -> p k c", k=NK, c=C),
                in_=w_in[nm].rearrange("k p c -> p k c"))
            wt[nm] = t

        xdram = {0: xf_d, 1: xe_d}
        xt_all = {}           # (im, k) -> tile;  im = b*2 + br

        def load_x(b, br, quarters):
            im = b * 2 + br
            for k in range(NK):
                xt_all[(im, k)] = xpool.tile(
                    [128, PIX], BF16, name=f"x_{im}_{k}", tag="x")
            if quarters:
                for s in range(2):
                    sl = slice(s * 2048, (s + 1) * 2048)
                    for k in range(NK):
                        nc.sync.dma_start(out=xt_all[(im, k)][:, sl],
                                          in_=xdram[br][b, k][:, sl])
            else:
                for k in range(NK):
                    nc.sync.dma_start(out=xt_all[(im, k)][:, :],
                                      in_=xdram[br][b, k])

        # wrf m0-columns first: the very first matmul needs only them
        wrf_t = cpool.tile([128, 2 * C], BF16, name="sb_wrf", tag="sb_wrf")
        wrf3 = wrf_t.rearrange("p (k c) -> p k c", k=NK, c=C)
        win3 = w_in["wrf"].rearrange("k p c -> p k c")
        nc.sync.dma_start(out=wrf3[:, :, 0:128], in_=win3[:, :, 0:128])
        wt["wrf"] = wrf_t
        load_x(0, 0, quarters=True)       # im0
        nc.sync.dma_start(out=wrf3[:, :, 128:256], in_=win3[:, :, 128:256])
        load_x(1, 0, quarters=False)      # im2
        load_w("wre", BF16)
        load_x(0, 1, quarters=False)      # im1
        load_x(1, 1, quarters=False)      # im3
        load_w("wke", F32)
        bk_sb = {}
        for nm, d in [("bkf", bkf_d), ("bke", bke_d)]:
            t = cpool.tile([128, 2], F32, name=f"sb_{nm}", tag=f"sb_{nm}")
            nc.sync.dma_start(out=t[:, :], in_=d[:, :])
            bk_sb[nm] = t
        packs = {}
        for nm, d in [("g1p", g1p_d), ("be1p", be1p_d), ("g2p", g2p_d),
                      ("be2p", be2p_d)]:
            t = cpool.tile([128, 4], F32, name=f"sb_{nm}", tag=f"sb_{nm}")
            nc.sync.dma_start(out=t[:, :], in_=d[:, :])
            packs[nm] = t
        ident = cpool.tile([128, 128], BF16, name="sb_ident", tag="sb_ident")
        nc.sync.dma_start(out=ident[:, :], in_=id_d[:, :])
        load_w("wkf", F32)
        load_w("wbf", BF16)
        load_w("wbe", BF16)

        pooled = {}
        kers = {}
        for b in range(BL):
            for br in range(2):
                for k in range(NK):
                    pooled[(b, br, k)] = spool.tile(
                        [128, 16], F32, name=f"pool_{b}_{br}_{k}", tag="pooled",
                        bufs=BL * 2 * NK)
                    kers[(b, br, k)] = spool.tile(
                        [128, 16], F32, name=f"ker_{b}_{br}_{k}", tag="kers",
                        bufs=BL * 2 * NK)

        y1 = {}
        y2 = {}

        # ---- pooling: 16x16 block sums, all DVE. One bf16 pairwise fold
        # at the 2x perf mode (error ~0.2% on pooled sums, negligible
        # after the kernel-generator averaging), then exact fp32 reduces.
        def do_pool(b, br, k):
            im = b * 2 + br
            xt = xt_all[(im, k)]
            x3 = xt.rearrange("p (yx xi) -> p yx xi", yx=256, xi=16)
            t1 = spool.tile([128, 2048], BF16, name=f"t1_{im}_{k}",
                            tag="poolt1", bufs=1)
            t13 = t1.rearrange("p (yx xh) -> p yx xh", yx=256, xh=8)
            nc.vector.tensor_tensor(out=t13, in0=x3[:, :, 0:8],
                                    in1=x3[:, :, 8:16], op=OP.add)
            s1 = spool.tile([128, 256], F32, name=f"s1_{im}_{k}",
                            tag="s1", bufs=2)
            nc.vector.tensor_reduce(
                out=s1[:, :], in_=t13, axis=mybir.AxisListType.X, op=OP.add)
            # s1 layout [(y64, xb4)]; reduce y-inner 16 (strided)
            s2in = s1.rearrange("p (yb yi xb) -> p yb xb yi", yb=4, yi=16,
                                xb=4)
            nc.vector.tensor_reduce(
                out=pooled[(b, br, k)].rearrange(
                    "p (yb xb) -> p yb xb", yb=4, xb=4),
                in_=s2in, axis=mybir.AxisListType.X, op=OP.add)

        def do_kergen(b, br):
            knm = "wkf" if br == 0 else "wke"
            bnm = "bkf" if br == 0 else "bke"
            for m in range(NK):
                kps = pspool.tile([128, 1024], F32, name=f"kgp_{b}_{br}_{m}",
                                  tag="dynps", bufs=2)
                for k in range(NK):
                    nc.tensor.matmul(
                        kps[:, 0:16],
                        wt[knm][:, k * C + m * 128:k * C + (m + 1) * 128],
                        pooled[(b, br, k)][:, :],
                        start=(k == 0), stop=(k == NK - 1))
                # bias on DVE: the ACT queue is deep in y1 evicts here and
                # kers gate the diag builds
                nc.vector.tensor_scalar(
                    out=kers[(b, br, m)][:, :], in0=kps[:, 0:16],
                    scalar1=bk_sb[bnm][:, m:m + 1], scalar2=None, op0=OP.add)

        # ---- conv_r / conv_b: PE matmuls + ACT evict w/ accum sums.
        # sumsq reads the PSUM tile directly on DVE, in parallel with the
        # ACT eviction, so branch stats complete right after the last
        # matmul instead of serializing behind the bf16 eviction. ----
        def conv1x1_pe(im, src, wnm, ydict, scr, ssq=None, ssq_psum=False):
            b = im // 2
            for m in range(NK):
                yt = imgpool.tile([128, PIX], BF16, name=f"y_{wnm}_{im}_{m}",
                                  tag="img")
                ydict[(im, m)] = yt
                for q in range(4):
                    mp = pspool.tile([128, 1024], F32,
                                     name=f"mp_{wnm}_{im}_{m}_{q}", tag="mmps",
                                     bufs=2)
                    for n in range(2):
                        off = q * 1024 + n * 512
                        for k in range(NK):
                            nc.tensor.matmul(
                                mp[:, n * 512:(n + 1) * 512],
                                wt[wnm][:, k * C + m * 128:
                                         k * C + (m + 1) * 128],
                                src[k][:, off:off + 512],
                                start=(k == 0), stop=(k == NK - 1))
                    g = q * 4 + b * 2 + m
                    sl = slice(q * 1024, (q + 1) * 1024)
                    nc.scalar.activation(
                        yt[:, sl], mp[:, :], AF.Copy,
                        accum_out=scr[:, g:g + 1])
                    if ssq is not None:
                        jk = scrpool.tile([128, 1024], BF16,
                                          name=f"jkp_{wnm}_{im}_{m}_{q}",
                                          tag="scr")
                        sqin = mp[:, :] if ssq_psum else yt[:, sl]
                        nc.vector.scalar_tensor_tensor(
                            out=jk[:, :], in0=sqin, scalar=1.0,
                            in1=sqin, op0=OP.mult, op1=OP.mult,
                            accum_out=ssq[:, g:g + 1])

        def sumsq_ops(im, yt_dict, scr_ssq, eng="dve"):
            b = im // 2
            for m in range(NK):
                yt = yt_dict[(im, m)]
                for q in range(4):
                    g = q * 4 + b * 2 + m
                    sl = slice(q * 1024, (q + 1) * 1024)
                    jk = scrpool.tile([128, 1024], BF16,
                                      name=f"jk_{id(scr_ssq)}_{im}_{m}_{q}",
                                      tag="scr")
                    # AP-scalar ops (TensorScalarPtr/STT) are DVE-only on
                    # the real ISA; the Pool engine rejects them.
                    nc.vector.scalar_tensor_tensor(
                        out=jk[:, :], in0=yt[:, sl], scalar=1.0,
                        in1=yt[:, sl], op0=OP.mult, op1=OP.mult,
                        accum_out=scr_ssq[:, g:g + 1])

        # ---- stat ladders (DVE) + collective (cin/gth on SP, cc on Pool) --
        def reduce16_to4(t16, dst, pfx):
            s8 = spool.tile([128, 8], F32, name=f"s8{pfx}", tag=f"s8{pfx}")
            nc.gpsimd.tensor_tensor(out=s8[:, :], in0=t16[:, 0:8],
                                    in1=t16[:, 8:16], op=OP.add)
            nc.gpsimd.tensor_tensor(out=dst, in0=s8[:, 0:4],
                                    in1=s8[:, 4:8], op=OP.add)

        def reduce_stats(scr, ssq, pfx):
            sums = spool.tile([128, 4], F32, name=f"sums{pfx}", tag=f"sums{pfx}")
            reduce16_to4(scr, sums[:, :], f"a{pfx}")
            sq4 = spool.tile([128, 4], F32, name=f"sq4{pfx}", tag=f"sq4{pfx}")
            reduce16_to4(ssq, sq4[:, :], f"b{pfx}")
            loc = spool.tile([128, 4], F32, name=f"loc{pfx}", tag=f"loc{pfx}")
            nc.gpsimd.tensor_tensor(out=loc[:, 0:2], in0=sums[:, 0:2],
                                    in1=sums[:, 2:4], op=OP.add)
            nc.gpsimd.tensor_tensor(out=loc[:, 2:4], in0=sq4[:, 0:2],
                                    in1=sq4[:, 2:4], op=OP.add)
            return loc

        cc_state = {}

        def cc_launch(pfx, loc):
            """stage local stats to DRAM (SP queue) + AllGather (Pool)."""
            cin = drpool.tile([128, 4], F32, name=f"{pfx}i", tag=f"{pfx}i")
            cout = drpool.tile([N_CORES, 128, 4], F32, name=f"{pfx}o",
                               tag=f"{pfx}o", addr_space="Shared")
            nc.sync.dma_start(out=cin[:, :], in_=loc[:, :])
            nc.gpsimd.collective_compute(
                "AllGather", OP.bypass,
                replica_groups=[list(range(N_CORES))],
                ins=[cin[:, :]], outs=[cout[:, :, :]])
            cc_state[pfx] = cout

        def cc_finish(pfx):
            """gather back (SP) + pairwise reduce over cores (DVE)."""
            cout = cc_state[pfx]
            gth = spool.tile([128, 32], F32, name=f"{pfx}g", tag=f"{pfx}g")
            nc.sync.dma_start(
                out=gth.rearrange("p (n f) -> p n f", n=N_CORES, f=4),
                in_=cout.rearrange("n p f -> p n f"))
            t16 = spool.tile([128, 16], F32, name=f"{pfx}h", tag=f"{pfx}h")
            nc.gpsimd.tensor_tensor(out=t16[:, :], in0=gth[:, 0:16],
                                    in1=gth[:, 16:32], op=OP.add)
            t8 = spool.tile([128, 8], F32, name=f"{pfx}q", tag=f"{pfx}q")
            nc.gpsimd.tensor_tensor(out=t8[:, :], in0=t16[:, 0:8],
                                    in1=t16[:, 8:16], op=OP.add)
            g = spool.tile([128, 4], F32, name=f"{pfx}r", tag=f"{pfx}r")
            nc.gpsimd.tensor_tensor(out=g[:, :], in0=t8[:, 0:4],
                                    in1=t8[:, 4:8], op=OP.add)
            return g

        def bn_coeffs(gst, gpack, bepack, pfx, eng="pool"):
            """global (sum, sumsq) [128,4] -> scale/shift [128,2]; Pool by
            default (off the contended DVE), DVE for the tail where DVE is
            idle. reciprocal + ACT Sqrt + one Newton step refine rsqrt."""
            ve = nc.gpsimd if eng == "pool" else nc.vector
            mean = spool.tile([128, 2], F32, name=f"{pfx}_mean", tag=f"{pfx}_mean")
            ve.tensor_scalar(out=mean[:, :], in0=gst[:, 0:2],
                             scalar1=1.0 / NTOT, scalar2=None, op0=OP.mult)
            vpe = spool.tile([128, 2], F32, name=f"{pfx}_vpe", tag=f"{pfx}_vpe")
            ve.tensor_scalar(out=vpe[:, :], in0=gst[:, 2:4],
                             scalar1=1.0 / NTOT, scalar2=EPS,
                             op0=OP.mult, op1=OP.add)
            msq = spool.tile([128, 2], F32, name=f"{pfx}_msq", tag=f"{pfx}_msq")
            ve.tensor_tensor(out=msq[:, :], in0=mean[:, :],
                             in1=mean[:, :], op=OP.mult)
            ve.tensor_tensor(out=vpe[:, :], in0=vpe[:, :],
                             in1=msq[:, :], op=OP.subtract)
            rcp = spool.tile([128, 2], F32, name=f"{pfx}_rcp", tag=f"{pfx}_rcp")
            nc.vector.reciprocal(rcp[:, :], vpe[:, :])
            r0 = spool.tile([128, 2], F32, name=f"{pfx}_r0", tag=f"{pfx}_r0")
            nc.scalar.activation(r0[:, :], rcp[:, :], AF.Sqrt)
            sc = spool.tile([128, 2], F32, name=f"{pfx}_sc", tag=f"{pfx}_sc")
            ve.tensor_tensor(out=sc[:, :], in0=gpack[:, :],
                             in1=r0[:, :], op=OP.mult)
            sh = spool.tile([128, 2], F32, name=f"{pfx}_sh", tag=f"{pfx}_sh")
            ve.tensor_tensor(out=sh[:, :], in0=mean[:, :],
                             in1=sc[:, :], op=OP.mult)
            ve.tensor_tensor(out=sh[:, :], in0=bepack[:, :],
                             in1=sh[:, :], op=OP.subtract)
            return sc, sh

        # ================= PHASE A =======================================
        scrA = {}
        ssqA = {}
        for br in range(2):
            scrA[br] = spool.tile([128, 16], F32, name=f"scrA{br}", tag=f"scrA{br}")
            ssqA[br] = spool.tile([128, 16], F32, name=f"ssqA{br}", tag=f"ssqA{br}")

        # br0 stats chain gates the first collective: split its sumsq over
        # DVE+Pool and launch cc10 as soon as im0/im2 y1 evicts land.
        # im2's pooling + br0 kergen are deferred into phase B (kers_f are
        # first needed by im1's diags, much later); br1's pooling runs
        # right after the cc10 launch so kergen_e -> im0 diags unblock.
        for k in range(NK):
            do_pool(0, 0, k)
        conv1x1_pe(0, {k: xt_all[(0, k)] for k in range(NK)},
                   "wrf", y1, scrA[0], ssq=ssqA[0])
        conv1x1_pe(2, {k: xt_all[(2, k)] for k in range(NK)},
                   "wrf", y1, scrA[0], ssq=ssqA[0])
        cc_launch("cc10", reduce_stats(scrA[0], ssqA[0], "A0"))

        for b in range(BL):
            for k in range(NK):
                do_pool(b, 1, k)
        conv1x1_pe(1, {k: xt_all[(1, k)] for k in range(NK)},
                   "wre", y1, scrA[1])
        conv1x1_pe(3, {k: xt_all[(3, k)] for k in range(NK)},
                   "wre", y1, scrA[1])
        for b in range(BL):
            do_kergen(b, 1)

        # ================= PHASE B =======================================
        outdram = {0: gf_d, 1: ge_d}
        sc1 = {}
        sh1 = {}
        sc2 = {}
        sh2 = {}

        def make_pad(b, br, k):
            """zero-bordered BN1+ReLU image chunk; DVE affine+relu interior,
            Pool border memsets."""
            im = b * 2 + br
            pt = padpool.tile([128, HP * WP], BF16, name=f"pad_{im}_{k}",
                              tag="pad")
            p3 = pt.rearrange("p (h w) -> p h w", h=HP, w=WP)
            nc.gpsimd.memset(p3[:, 0:2, :], 0.0)      # top 2 rows
            nc.gpsimd.memset(p3[:, 66:67, :], 0.0)    # bottom row
            nc.gpsimd.memset(p3[:, 2:66, 0:2], 0.0)   # left 2 cols
            nc.gpsimd.memset(p3[:, 2:66, 66:68], 0.0)  # right 2 cols
            # two row bands: the first dyn matmuls start after band A
            y3 = y1[(im, k)].rearrange("p (h w) -> p h w", h=64, w=64)
            for r0, r1 in ((0, 33), (33, 64)):
                band = p3[:, 2 + r0:2 + r1, 2:66]
                nc.vector.tensor_scalar(
                    out=band, in0=y3[:, r0:r1, :],
                    scalar1=sc1[br][:, k:k + 1], scalar2=sh1[br][:, k:k + 1],
                    op0=OP.mult, op1=OP.add)
                nc.vector.tensor_scalar(
                    out=band, in0=band, scalar1=0.0, scalar2=None,
                    op0=OP.max)
            return p3

        def make_diags(b, br, k, taps=range(16)):
            """diagonal tap matrices from the OTHER branch's kernels (DVE)."""
            im = b * 2 + br
            kt = kers[(b, 1 - br, k)]
            dg = {}
            for t in taps:
                dt_ = dpool.tile([128, 128], BF16, name=f"dg_{im}_{k}_{t}",
                                 tag="diag")
                nc.vector.tensor_scalar(
                    out=dt_[:, :], in0=ident[:, :],
                    scalar1=kt[:, t:t + 1], scalar2=None, op0=OP.mult)
                dg[t] = dt_
            return dg

        def dyn_pe(im, k, p3, dg, taps=range(16)):
            """grouped conv on PE: diag matmuls over shifted APs."""
            gt = gpool.tile([128, PIX], BF16, name=f"gd_{im}_{k}", tag="guide")
            taps = list(taps)
            for q in range(4):
                dp = pspool.tile([128, 1024], F32, name=f"dp_{im}_{k}_{q}",
                                 tag="dynps", bufs=2)
                for t in taps:
                    i, j = t // 4, t % 4
                    for n in range(2):
                        r0_ = q * 16 + n * 8 + i
                        nc.tensor.matmul(
                            dp[:, n * 512:(n + 1) * 512],
                            dg[t][:, :],
                            p3[:, r0_:r0_ + 8, j:j + 64],
                            start=(t == taps[0]), stop=(t == taps[-1]))
                nc.scalar.activation(
                    gt[:, q * 1024:(q + 1) * 1024], dp[:, :], AF.Copy)
            return gt

        def dyn_dve(im, k, p3, b, br, taps=range(16)):
            """grouped-conv tap subset as a DVE FMA chain (1x, slow — use
            only to shave the PE critical path with idle DVE time)."""
            kt = kers[(b, 1 - br, k)]
            taps = list(taps)
            gt = gpool.tile([128, PIX], BF16, name=f"gdv_{im}_{k}",
                            tag="guide")
            g3 = gt.rearrange("p (h w) -> p h w", h=64, w=64)
            for t in taps:
                i, j = t // 4, t % 4
                win = p3[:, i:i + 64, j:j + 64]
                if t == taps[0]:
                    nc.vector.tensor_scalar(
                        out=g3, in0=win, scalar1=kt[:, t:t + 1], scalar2=None,
                        op0=OP.mult)
                else:
                    nc.vector.scalar_tensor_tensor(
                        out=g3, in0=win, scalar=kt[:, t:t + 1], in1=g3,
                        op0=OP.mult, op1=OP.add)
            return gt

        def final_apply(b, br, m, engine):
            """BN2+ReLU -> bf16 -> DMA out, quartered for DMA pipelining."""
            im = b * 2 + br
            ot = opool.tile([128, PIX], BF16, name=f"o_{im}_{m}", tag="outb")
            for h in range(2):
                sl = slice(h * 2048, (h + 1) * 2048)
                if engine == "dve":
                    nc.vector.tensor_scalar(
                        out=ot[:, sl], in0=y2[(im, m)][:, sl],
                        scalar1=sc2[br][:, m:m + 1],
                        scalar2=sh2[br][:, m:m + 1],
                        op0=OP.mult, op1=OP.add)
                    nc.vector.tensor_scalar(
                        out=ot[:, sl], in0=ot[:, sl], scalar1=0.0,
                        scalar2=None, op0=OP.max)
                else:
                    nc.scalar.activation(
                        ot[:, sl], y2[(im, m)][:, sl], AF.Relu,
                        bias=sh2[br][:, m:m + 1],
                        scale=sc2[br][:, m:m + 1])
                nc.sync.dma_start(out=outdram[br][b, m][:, sl],
                                  in_=ot[:, sl])

        scrB = {}
        ssqB = {}
        for br in range(2):
            scrB[br] = spool.tile([128, 16], F32, name=f"scrB{br}", tag=f"scrB{br}")
            ssqB[br] = spool.tile([128, 16], F32, name=f"ssqB{br}", tag=f"ssqB{br}")

        def image_phaseB(b, br, dg0=None, ssq_psum=False):
            """pads + dyn + conv_b for one image."""
            im = b * 2 + br
            guides = {}
            for k in range(NK):
                p3 = make_pad(b, br, k)
                if (im, k) in HALF_DVE:
                    # leading taps on PE, trailing taps on DVE, one-add merge
                    nd = HALF_DVE[(im, k)]
                    dg = make_diags(b, br, k, taps=range(16 - nd))
                    ga = dyn_pe(im, k, p3, dg, taps=range(16 - nd))
                    gb = dyn_dve(im, k, p3, b, br, taps=range(16 - nd, 16))
                    nc.vector.tensor_tensor(out=ga[:, :], in0=ga[:, :],
                                            in1=gb[:, :], op=OP.add)
                    guides[k] = ga
                elif (im, k) in DVE_DYN:
                    guides[k] = dyn_dve(im, k, p3, b, br)
                else:
                    dg = dg0 if (k == 0 and dg0 is not None) \
                        else make_diags(b, br, k)
                    guides[k] = dyn_pe(im, k, p3, dg)
            bnm2 = "wbf" if br == 0 else "wbe"
            conv1x1_pe(im, guides, bnm2, y2, scrB[br], ssq=ssqB[br],
                       ssq_psum=ssq_psum)

        # diags for im0 k0 ahead of the BN1-coeff wait on the DVE queue
        # (they depend only on br1 kers)
        dg00 = make_diags(0, 0, 0)

        gst1_0 = cc_finish("cc10")
        sc1[0], sh1[0] = bn_coeffs(
            gst1_0, packs["g1p"][:, 0:2], packs["be1p"][:, 0:2], "c10")

        # ---- im0 (b0, br0) ----
        image_phaseB(0, 0, dg0=dg00)

        # br1's deferred phase-A stats: DVE is free here, after im0's pads
        sumsq_ops(1, y1, ssqA[1])
        sumsq_ops(3, y1, ssqA[1])
        cc_launch("cc11", reduce_stats(scrA[1], ssqA[1], "A1"))

        # deferred im2 pooling + br0 kergen (kers_f first used by im1)
        for k in range(NK):
            do_pool(1, 0, k)
        for b in range(BL):
            do_kergen(b, 0)

        # ---- im2 (b1, br0) ----
        image_phaseB(1, 0)

        gst1_1 = cc_finish("cc11")
        sc1[1], sh1[1] = bn_coeffs(
            gst1_1, packs["g1p"][:, 2:4], packs["be1p"][:, 2:4], "c11")

        # ---- im1 (b0, br1) ----
        image_phaseB(0, 1)
        cc_launch("cc20", reduce_stats(scrB[0], ssqB[0], "B0"))

        # ---- im3 (b1, br1): sumsq straight from PSUM — the stats gate
        # the tail collective and nothing runs after to be throttled ----
        image_phaseB(1, 1)

        # ---- br0 finals (all DVE; ACT busy trailing br1 evicts) ----
        gst2_0 = cc_finish("cc20")
        sc2[0], sh2[0] = bn_coeffs(
            gst2_0, packs["g2p"][:, 0:2], packs["be2p"][:, 0:2], "c20")
        for b in range(BL):
            for m in range(NK):
                final_apply(b, 0, m, "dve")

        cc_launch("cc21", reduce_stats(scrB[1], ssqB[1], "B1"))

        # ---- br1 finals (split ACT/DVE for a short tail) ----
        gst2_1 = cc_finish("cc21")
        sc2[1], sh2[1] = bn_coeffs(
            gst2_1, packs["g2p"][:, 2:4], packs["be2p"][:, 2:4], "c21",
            eng="dve")
        final_apply(0, 1, 0, "dve")
        final_apply(0, 1, 1, "act")
        final_apply(1, 1, 0, "dve")
        final_apply(1, 1, 1, "dve")

    nc.compile()
    return nc


def _prep_maps(xf, xe, w_kf, b_kf, w_ke, b_ke, w_rf, g_rf, be_rf, w_re, g_re,
               be_re, w_bf, g_bf, be_bf, w_be, g_be, be_be):
    bf = ml_dtypes.bfloat16
    common = {}
    for nm, w, dt_ in [("wrf", w_rf, bf), ("wre", w_re, bf), ("wbf", w_bf, bf),
                       ("wbe", w_be, bf), ("wkf", w_kf / 256.0, np.float32),
                       ("wke", w_ke / 256.0, np.float32)]:
        wT = np.ascontiguousarray(np.asarray(w, np.float32).T.astype(dt_))
        common[nm] = wT.reshape(NK, 128, C)
    common["bkf"] = np.ascontiguousarray(
        np.asarray(b_kf, np.float32).reshape(2, 128).T)
    common["bke"] = np.ascontiguousarray(
        np.asarray(b_ke, np.float32).reshape(2, 128).T)

    def pack(gf_, ge_):
        p = np.zeros((128, 4), np.float32)
        for br in range(2):
            for m in range(NK):
                v = gf_ if br == 0 else ge_
                p[:, br * 2 + m] = np.asarray(v, np.float32)[
                    m * 128:(m + 1) * 128]
        return p

    common["g1p"] = pack(g_rf, g_re)
    common["be1p"] = pack(be_rf, be_re)
    common["g2p"] = pack(g_bf, g_be)
    common["be2p"] = pack(be_bf, be_be)
    common["identbf"] = np.eye(128, dtype=np.float32).astype(bf)

    xf = np.asarray(xf, np.float32).reshape(N_CORES, BL, NK, 128, PIX)
    xe = np.asarray(xe, np.float32).reshape(N_CORES, BL, NK, 128, PIX)
    maps = []
    for c in range(N_CORES):
        m = dict(common)
        m["xf"] = xf[c].astype(bf)
        m["xe"] = xe[c].astype(bf)
        maps.append(m)
    return maps


def kernel(xf, xe, w_kf, b_kf, w_ke, b_ke,
           w_rf, b_rf, g_rf, be_rf, w_re, b_re, g_re, be_re,
           w_bf, b_bf, g_bf, be_bf, w_be, b_be, g_be, be_be):
    # note: conv biases feeding a train-mode BatchNorm cancel exactly
    # (BN subtracts the batch mean), so b_rf/b_re/b_bf/b_be are unused.
    try:
        import jax
        jax.config.update("jax_compilation_cache_dir", "/tmp/jaxcache_kernel")
        jax.config.update("jax_persistent_cache_min_entry_size_bytes", 0)
        jax.config.update("jax_persistent_cache_min_compile_time_secs", 0)
    except Exception:
        pass
    if "nc" not in _CACHE:
        _CACHE["nc"] = build()
    nc = _CACHE["nc"]
    maps = _prep_maps(xf, xe, w_kf, b_kf, w_ke, b_ke, w_rf, g_rf, be_rf,
                      w_re, g_re, be_re, w_bf, g_bf, be_bf, w_be, g_be, be_be)
    res = run_bass_kernel_spmd(nc, maps, core_ids=list(range(N_CORES)))
    gf = np.concatenate(
        [np.asarray(r["gf"]).astype(np.float32).reshape(BL, C, H, W)
         for r in res.results])
    ge = np.concatenate(
        [np.asarray(r["ge"]).astype(np.float32).reshape(BL, C, H, W)
         for r in res.results])
    return gf, ge
